# revision 3
# baseline (speedup 1.0000x reference)
"""Causal self-attention (B=4, T=4096, D=768, single head, fp32) on 8 TRN2
NeuronCores — v2: projection-free attention via reassociation.

Sharding: core <-> (batch b = core//2, parity h = core%2) as v1: local q-tile
i = 0..15 maps to global q-tile g = 2i + h; per local q-tile i the kernel
processes keys [0, 256*(i+1)) in 8 superblocks of 512.

Key algebraic changes vs v1 (remove the per-core duplicated K/V projections):
  - K-trick: S = Q.K^T = (X.Wq^T)(X.Wk^T)^T = (X.W').X^T with W' = Wq^T.Wk.
    W' is computed once on device (72 small matmuls), Q' = X.W' is projected
    once for the local q columns, and S^T = X.Q'^T uses the streamed x^T
    chunks directly as the stationary operand. The per-superblock K
    projection (61us/core) is gone.
  - V-trick: O = P.V = P.(X.Wv^T) = (P.[X|1]).Wv^T. The P-matmul accumulates
    Z = P.[X|1] (same cost as P.V, moving operand is DMA'd x rows instead of
    projected V), and at q-tile retirement Z is transposed on the PE (6
    is_transpose matmuls vs identity) and projected through Wv^T once. The
    per-superblock V projection (61us/core) is gone; l comes from the ones
    column exactly as v1.
  - Causal skip: terminal q-tile 2sb only consumes kv-tiles {0,1} of its
    superblock in the P-matmul (tiles 2,3 are fully masked for both
    parities), so their P.[X|1] matmuls and mask fixups are dropped.

Everything else (fp32r matmuls, no-max softmax, [kv,q]-layout S^T so exp
output feeds P-matmuls stationary without transposes, PSUM accumulation per
q-tile, progressive retirement) follows v1.
"""

import os
import sys
from contextlib import ExitStack

import numpy as np

if "/opt/trn_rl_repo" not in sys.path:
    sys.path.insert(0, "/opt/trn_rl_repo")

B, T, D = 4, 4096, 768
N_CORES = 8
QTILES = 16          # local q-tiles per core, 128 rows each
EC = D // 128        # 6 d/e chunks of 128
SB = 8               # kv superblocks
SBW = 512            # superblock width (keys)
NKT = SBW // 128     # kv 128-tiles per superblock
OSTR = D + 4         # oacc slot stride: [Z 768 | l 1 | pad 3]
XRW = D + 4          # xr row-tile width: [X 768 | ones 2 | pad 2]
NEG = -1.0e9
SCALE = 1.0 / float(np.sqrt(D))

_CACHE = {}


def _patch_tile_drain():
    """This walrus build accepts only one sync wait per instruction;
    TileContext's tail drain carries one wait per outstanding proc. Split
    them onto individual SP no-ops (SP executes sequentially, so semantics
    are unchanged)."""
    import concourse.mybir as mybir
    import concourse.tile as tile
    from concourse.vector_clock import ScopedClock

    if getattr(tile.TileContext, "_drain_split_patch", False):
        return

    def _split_drain_and_barrier(self, tick_clock, wait_clock):
        nc = self.nc
        carrier = nc.sync.nop(nofuse=True)
        wait_clock.add_sem_waits(
            carrier.ins, ScopedClock({None: tick_clock.global_clock})
        )
        si = carrier.ins.sync_info
        waits = list(si.on_wait) if si is not None else []
        carrier.ins.sync_info = mybir.SyncInfo(on_wait=waits[:1], on_update=[])
        for w in waits[1:]:
            n = nc.sync.nop(nofuse=True)
            n.ins.sync_info = mybir.SyncInfo(on_wait=[w], on_update=[])
        nc.sync.drain()
        nc.all_engine_barrier()
        assert self.sems is not None
        popped = nc._tile_sem_poison_stack.pop()
        assert popped is self._sem_poison
        nc.clear_and_free_semaphores(list(self.sems.allocated().values()))
        nc.all_engine_barrier()

    tile.TileContext._drain_and_barrier = _split_drain_and_barrier
    tile.TileContext._drain_split_patch = True


def _hoist_multi_waits(nc):
    """This walrus build encodes at most ONE sync wait per instruction
    descriptor. Tile's sem assignment can put several waits on one
    instruction; hoist the extras onto same-engine no-ops inserted
    immediately before it — the engine executes them sequentially, so the
    wait semantics are unchanged."""
    import concourse.mybir as mybir

    n = 0
    for fn in nc.m.functions:
        for bb in fn.blocks:
            insts = bb.instructions
            out = []
            for ins in insts:
                si = ins.sync_info
                waits = list(si.on_wait) if si is not None else []
                if len(waits) > 1:
                    for w in waits[:-1]:
                        nop = mybir.InstNoOp(
                            name=f"I-hoistw-{nc.next_id()}",
                            engine=ins.engine,
                            ins=[],
                            outs=[],
                            sync_info=mybir.SyncInfo(on_wait=[w], on_update=[]),
                        )
                        out.append(nop)
                        n += 1
                    ins.sync_info = mybir.SyncInfo(
                        on_wait=[waits[-1]], on_update=list(si.on_update)
                    )
                out.append(ins)
            insts[:] = out
    return n


def _build_program(hoist=True):
    import concourse.bass as bass
    import concourse.mybir as mybir
    import concourse.tile as tile

    _patch_tile_drain()
    f32 = mybir.dt.float32
    f32r = mybir.dt.float32r
    Exp = mybir.ActivationFunctionType.Exp
    Copy = mybir.ActivationFunctionType.Copy

    nc = bass.Bass()
    xkvT = nc.dram_tensor("xkvT", [128, EC, T], f32r, kind="ExternalInput")
    xqT = nc.dram_tensor("xqT", [128, EC, QTILES * 128], f32r, kind="ExternalInput")
    xr = nc.dram_tensor("xr", [128, T // 128, D], f32r, kind="ExternalInput")
    wkR = nc.dram_tensor("wkR", [128, EC, D], f32r, kind="ExternalInput")
    wqR = nc.dram_tensor("wqR", [128, EC, D], f32r, kind="ExternalInput")
    wvT = nc.dram_tensor("wvT", [128, EC, D], f32r, kind="ExternalInput")
    ident = nc.dram_tensor("ident", [128, 128], f32, kind="ExternalInput")
    # masks are [kv, q] (transposed) here
    maska = nc.dram_tensor("maska", [128, 128], f32, kind="ExternalInput")
    maskb = nc.dram_tensor("maskb", [128, 128], f32, kind="ExternalInput")
    out_d = nc.dram_tensor("out", [QTILES * 128, D], f32, kind="ExternalOutput")

    with tile.TileContext(nc) as tc:
        with (
            tc.tile_pool(name="consts", bufs=1) as cpool,
            tc.tile_pool(name="qp", bufs=1) as qpool,
            tc.tile_pool(name="ps_st", bufs=4, space="PSUM") as ps_st,
            tc.tile_pool(name="ps_o", bufs=2, space="PSUM") as ps_o,
        ):
            # ---- W' = Wq^T.Wk on device, then Q'^T projection to SBUF ----
            # DMA order: W'-proj inputs first (first PE work), then the xq
            # stream it feeds, then constants needed later.
            qt_t = qpool.tile([128, EC, QTILES * 128], f32r, tag="qt")
            with tc.tile_pool(name="qproj", bufs=1) as qppool, \
                 tc.tile_pool(name="wpool", bufs=1) as wpool:
                wk_t = wpool.tile([128, EC, D], f32r, tag="wk")
                nc.sync.dma_start(out=wk_t[:], in_=wkR[:])
                wq_t = wpool.tile([128, EC, D], f32r, tag="wq")
                nc.sync.dma_start(out=wq_t[:], in_=wqR[:])
                xq_ts = []
                for qc in range(4):
                    xq_c = qppool.tile([128, EC, 512], f32r, tag=f"xq{qc}")
                    nc.sync.dma_start(
                        out=xq_c[:], in_=xqT[:, :, qc * 512 : (qc + 1) * 512]
                    )
                    xq_ts.append(xq_c)
                id_t = cpool.tile([128, 128], f32, tag="id")
                nc.sync.dma_start(out=id_t[:], in_=ident[:])
                ma_t = cpool.tile([128, 128], f32, tag="ma")
                nc.sync.dma_start(out=ma_t[:], in_=maska[:])
                mb_t = cpool.tile([128, 128], f32, tag="mb")
                nc.sync.dma_start(out=mb_t[:], in_=maskb[:])

                # W'[d, d'] = sum_e Wq[e, d] Wk[e, d'], chunk layout
                # w2[p, m, c] = W'[m*128+p, c]
                w2_t = qppool.tile([128, EC, D], f32r, tag="w2")
                for m in range(EC):
                    pw = ps_o.tile([128, 1024], f32, tag="o", name=f"psw{m}")
                    for j in range(EC):
                        for lo, n in ((0, 512), (512, 256)):
                            nc.tensor.matmul(
                                pw[:, lo : lo + n],
                                wq_t[:, j, m * 128 : (m + 1) * 128],
                                wk_t[:, j, lo : lo + n],
                                start=(j == 0),
                                stop=(j == EC - 1),
                            )
                    nc.scalar.copy(out=w2_t[:, m, :], in_=pw[:, :D])

                # Q'^T[d', q] per chunk m: stationary w2[:, j, m-block],
                # moving xq chunks. qc-outer so S^T g0 can start early.
                for qc in range(4):
                    for m in range(EC):
                        psq = ps_st.tile(
                            [128, 512], f32, tag="st", name=f"psq{qc}_{m}"
                        )
                        for j in range(EC):
                            nc.tensor.matmul(
                                psq[:],
                                w2_t[:, j, m * 128 : (m + 1) * 128],
                                xq_ts[qc][:, j, :],
                                start=(j == 0),
                                stop=(j == EC - 1),
                            )
                        nc.scalar.copy(
                            out=qt_t[:, m, qc * 512 : (qc + 1) * 512], in_=psq[:]
                        )

            # ---- kv superblocks ----
            # oacc and Wv^T live in pools opened after the projection pools
            # close, reusing their space (pool allocation is stack-ordered).
            attn_pools = ExitStack()
            oapool = attn_pools.enter_context(tc.tile_pool(name="oacc", bufs=1))
            oacc_ts = [
                oapool.tile([128, OSTR], f32, tag=f"oacc{i}", name=f"oacc{i}")
                for i in range(QTILES)
            ]
            wv_t = oapool.tile([128, EC, D], f32r, tag="wv")
            nc.sync.dma_start(out=wv_t[:], in_=wvT[:])
            xspool = attn_pools.enter_context(tc.tile_pool(name="xs", bufs=2))
            xrpool = attn_pools.enter_context(tc.tile_pool(name="xr", bufs=2))
            ptpool = attn_pools.enter_context(tc.tile_pool(name="pt", bufs=2))
            ztpool = attn_pools.enter_context(tc.tile_pool(name="zt", bufs=2))
            spool = attn_pools.enter_context(tc.tile_pool(name="small", bufs=2))
            obpool = attn_pools.enter_context(tc.tile_pool(name="ob", bufs=2))
            def emit_retire(sb, i):
                # O = (Z.Wv^T) / l: Z^T chunks via PE transpose, then the
                # Wv^T projection, normalize on the way out.
                zt_t = ztpool.tile([128, D], f32r, tag="zt")
                for j in range(EC):
                    pz = ps_st.tile(
                        [128, 512], f32, tag="st", name=f"pz{sb}_{i}_{j}"
                    )
                    nc.tensor.transpose(
                        pz[:, 0:128],
                        oacc_ts[i][:, j * 128 : (j + 1) * 128],
                        id_t[:],
                    )
                    if j % 2 == 0:
                        nc.vector.tensor_copy(
                            out=zt_t[:, j * 128 : (j + 1) * 128],
                            in_=pz[:, 0:128],
                        )
                    else:
                        nc.scalar.copy(
                            out=zt_t[:, j * 128 : (j + 1) * 128],
                            in_=pz[:, 0:128],
                        )
                po2 = ps_o.tile([128, 1024], f32, tag="o", name=f"po2_{sb}_{i}")
                for j in range(EC):
                    for lo, n in ((0, 512), (512, 256)):
                        nc.tensor.matmul(
                            po2[:, lo : lo + n],
                            zt_t[:, j * 128 : (j + 1) * 128],
                            wv_t[:, j, lo : lo + n],
                            start=(j == 0),
                            stop=(j == EC - 1),
                        )
                recip = spool.tile([128, 1], f32, tag="recip")
                nc.vector.reciprocal(
                    out=recip[:], in_=oacc_ts[i][:, D : D + 1]
                )
                ob = obpool.tile([128, D], f32, tag="ob")
                nc.scalar.activation(ob[:], po2[:, :D], Copy, scale=recip[:, 0:1])
                nc.sync.dma_start(
                    out=out_d[i * 128 : (i + 1) * 128, :], in_=ob[:]
                )

            for sb in range(SB):
                xkv_t = xspool.tile([128, EC, SBW], f32r, tag="xs")
                nc.sync.dma_start(
                    out=xkv_t[:], in_=xkvT[:, :, sb * SBW : (sb + 1) * SBW]
                )
                # x rows for this superblock, with a ones column for l
                xr_t = xrpool.tile([128, NKT, XRW], f32r, tag="xr")
                nc.sync.dma_start(
                    out=xr_t[:, :, :D],
                    in_=xr[:, sb * NKT : (sb + 1) * NKT, :],
                )
                nc.vector.memset(xr_t[:, :, D : D + 2].bitcast(f32), 1.0)

                # ---- attention, in q-groups of up to 512 columns ----
                # active q-tiles: i in [2*sb, 16); groups are 512-aligned
                i_lo = 2 * sb
                g_lo = i_lo // 4
                for g in range(g_lo, 4):
                    ia = max(i_lo, 4 * g)      # first active q-tile in group
                    ib = 4 * g + 4             # end q-tile (exclusive)
                    qc0 = ia * 128             # first active q column
                    gw = (ib - ia) * 128       # group width (256 or 512)

                    # S^T = X.Q'^T for the group's q span, per kv-tile:
                    # stationary is the streamed x^T chunk directly.
                    stg = [
                        ps_st.tile([128, 512], f32, tag="st", name=f"st{sb}_{g}_{k}")
                        for k in range(NKT)
                    ]
                    for kt in range(NKT):
                        # q-tile 2sb never consumes kv-tiles 2,3: drop its
                        # columns there when the remainder stays >= 256 wide
                        lo = 128 if (kt >= 2 and ia == 2 * sb and gw == 512) else 0
                        for j in range(EC):
                            nc.tensor.matmul(
                                stg[kt][:, lo:gw],
                                xkv_t[:, j, kt * 128 : (kt + 1) * 128],
                                qt_t[:, j, qc0 + lo : qc0 + gw],
                                start=(j == 0),
                                stop=(j == EC - 1),
                            )
                    # causal fixups for the terminal q-tiles of this sb:
                    # q-tile 2sb terminates at kv-tiles (0,1) of this sb
                    # (tiles 2,3 are dropped from its P-matmul); q-tile
                    # 2sb+1 terminates at kv-tiles (2,3).
                    for i, kts in ((2 * sb, ((0, ma_t), (1, mb_t))),
                                   (2 * sb + 1, ((2, ma_t), (3, mb_t)))):
                        if not (ia <= i < ib):
                            continue
                        qo = i * 128 - qc0
                        for kt, m in kts:
                            nc.vector.tensor_add(
                                stg[kt][:, qo : qo + 128],
                                stg[kt][:, qo : qo + 128],
                                m[:],
                            )
                    # P^T = exp(S^T * scale) back to SBUF
                    pt_t = ptpool.tile([128, NKT, 512], f32r, tag="pt")
                    for kt in range(NKT):
                        lo = 128 if (kt >= 2 and ia == 2 * sb and gw == 512) else 0
                        nc.scalar.activation(
                            pt_t[:, kt, lo:gw], stg[kt][:, lo:gw], Exp, scale=SCALE
                        )
                    # Z += P.[X|1] per active q-tile (kv-tiles 2,3 are fully
                    # masked for q-tile 2sb on both parities: skip them)
                    for i in range(ia, ib):
                        qo = i * 128 - qc0
                        kts = (0, 1) if i == 2 * sb else (0, 1, 2, 3)
                        po = ps_o.tile([128, 1024], f32, tag="o")
                        for ki, kt in enumerate(kts):
                            lhs = pt_t[:, kt, qo : qo + 128]
                            for lo, n in ((0, 512), (512, 258)):
                                nc.tensor.matmul(
                                    po[:, lo : lo + n],
                                    lhs,
                                    xr_t[:, kt, lo : lo + n],
                                    start=(ki == 0),
                                    stop=(ki == len(kts) - 1),
                                )
                        osl = oacc_ts[i][:, : D + 1]
                        if sb == 0:
                            nc.vector.tensor_copy(out=osl, in_=po[:, : D + 1])
                        else:
                            nc.vector.tensor_add(osl, po[:, : D + 1], osl)

                    # retire the terminal q-tiles as soon as their group's
                    # merges are in; spread the two retires across groups to
                    # avoid bursting the pz ring / copy engines
                    if g == g_lo:
                        emit_retire(sb, 2 * sb)
                    if g == min(g_lo + 1, 3):
                        emit_retire(sb, 2 * sb + 1)

            attn_pools.close()
    if hoist:
        _hoist_multi_waits(nc)
    return nc


def _prep_inputs(x, W_q, W_k, W_v):
    """Per-core input maps. Host-side work is layout only (transposes,
    slicing, mask construction)."""

    def chunked(a):  # [768, N] -> [128, EC, N]
        return np.ascontiguousarray(a.reshape(EC, 128, -1).transpose(1, 0, 2))

    wkRa = chunked(W_k)          # Wk rows e-chunked
    wqRa = chunked(W_q)          # Wq rows e-chunked
    wvTa = chunked(W_v.T.copy())  # Wv^T d-chunked
    ident = np.eye(128, dtype=np.float32)

    r = np.arange(128, dtype=np.float32)
    # [q, c] triangle: allowed iff c <= q; stored transposed ([kv, q])
    tri = np.where(r[None, :] <= r[:, None], 0.0, NEG).astype(np.float32)
    triT = np.ascontiguousarray(tri.T)
    zero = np.zeros((128, 128), dtype=np.float32)
    full = np.full((128, 128), NEG, dtype=np.float32)
    # per-parity (maska, maskb) for the terminal 256 kv columns
    masks_ab = [(triT, full), (zero, triT)]

    in_maps = []
    qsels = []
    for c in range(N_CORES):
        b, h = c // 2, c % 2
        xT = chunked(np.ascontiguousarray(x[b].T))  # [128, EC, T]
        xrows = np.ascontiguousarray(
            x[b].reshape(T // 128, 128, D).transpose(1, 0, 2)
        )
        qsel = np.concatenate(
            [np.arange((2 * i + h) * 128, (2 * i + h + 1) * 128) for i in range(QTILES)]
        )
        qsels.append(qsel)
        ma, mb = masks_ab[h]
        in_maps.append(
            {
                "xkvT": xT,
                "xqT": np.ascontiguousarray(xT[:, :, qsel]),
                "xr": xrows,
                "wkR": wkRa,
                "wqR": wqRa,
                "wvT": wvTa,
                "ident": ident,
                "maska": ma,
                "maskb": mb,
            }
        )
    return in_maps, qsels


def kernel(x, W_q, W_k, W_v, _trace=False):
    from concourse.bass_utils import run_bass_kernel_spmd

    if "nc" not in _CACHE:
        _CACHE["nc"] = _build_program()
    nc = _CACHE["nc"]

    in_maps, qsels = _prep_inputs(
        np.asarray(x, dtype=np.float32),
        np.asarray(W_q, dtype=np.float32),
        np.asarray(W_k, dtype=np.float32),
        np.asarray(W_v, dtype=np.float32),
    )
    res = run_bass_kernel_spmd(nc, in_maps, list(range(N_CORES)), trace=_trace)
    _CACHE["last_results"] = res

    out = np.empty((B, T, D), dtype=np.float32)
    for c in range(N_CORES):
        b = c // 2
        out[b, qsels[c]] = res.results[c]["out"]
    return out


# revision 4
# speedup vs baseline: 1.0234x; 1.0234x over previous
"""Causal self-attention (B=4, T=4096, D=768, single head, fp32) on 8 TRN2
NeuronCores — v2: projection-free attention via reassociation.

Sharding: core <-> (batch b = core//2, parity h = core%2) as v1: local q-tile
i = 0..15 maps to global q-tile g = 2i + h; per local q-tile i the kernel
processes keys [0, 256*(i+1)) in 8 superblocks of 512.

Key algebraic changes vs v1 (remove the per-core duplicated K/V projections):
  - K-trick: S = Q.K^T = (X.Wq^T)(X.Wk^T)^T = (X.W').X^T with W' = Wq^T.Wk.
    W' is computed once on device (72 small matmuls), Q' = X.W' is projected
    once for the local q columns, and S^T = X.Q'^T uses the streamed x^T
    chunks directly as the stationary operand. The per-superblock K
    projection (61us/core) is gone.
  - V-trick: O = P.V = P.(X.Wv^T) = (P.[X|1]).Wv^T. The P-matmul accumulates
    Z = P.[X|1] (same cost as P.V, moving operand is DMA'd x rows instead of
    projected V), and at q-tile retirement Z is transposed on the PE (6
    is_transpose matmuls vs identity) and projected through Wv^T once. The
    per-superblock V projection (61us/core) is gone; l comes from the ones
    column exactly as v1.
  - Causal skip: terminal q-tile 2sb only consumes kv-tiles {0,1} of its
    superblock in the P-matmul (tiles 2,3 are fully masked for both
    parities), so their P.[X|1] matmuls and mask fixups are dropped.

Everything else (fp32r matmuls, no-max softmax, [kv,q]-layout S^T so exp
output feeds P-matmuls stationary without transposes, PSUM accumulation per
q-tile, progressive retirement) follows v1.
"""

import os
import sys
from contextlib import ExitStack

import numpy as np

if "/opt/trn_rl_repo" not in sys.path:
    sys.path.insert(0, "/opt/trn_rl_repo")

B, T, D = 4, 4096, 768
N_CORES = 8
QTILES = 16          # local q-tiles per core, 128 rows each
EC = D // 128        # 6 d/e chunks of 128
SB = 8               # kv superblocks
SBW = 512            # superblock width (keys)
NKT = SBW // 128     # kv 128-tiles per superblock
OSTR = D + 4         # oacc slot stride: [Z 768 | l 1 | pad 3]
XRW = D + 4          # xr row-tile width: [X 768 | ones 2 | pad 2]
NEG = -1.0e9
SCALE = 1.0 / float(np.sqrt(D))

_CACHE = {}


def _patch_tile_drain():
    """This walrus build accepts only one sync wait per instruction;
    TileContext's tail drain carries one wait per outstanding proc. Split
    them onto individual SP no-ops (SP executes sequentially, so semantics
    are unchanged)."""
    import concourse.mybir as mybir
    import concourse.tile as tile
    from concourse.vector_clock import ScopedClock

    if getattr(tile.TileContext, "_drain_split_patch", False):
        return

    def _split_drain_and_barrier(self, tick_clock, wait_clock):
        nc = self.nc
        carrier = nc.sync.nop(nofuse=True)
        wait_clock.add_sem_waits(
            carrier.ins, ScopedClock({None: tick_clock.global_clock})
        )
        si = carrier.ins.sync_info
        waits = list(si.on_wait) if si is not None else []
        carrier.ins.sync_info = mybir.SyncInfo(on_wait=waits[:1], on_update=[])
        for w in waits[1:]:
            n = nc.sync.nop(nofuse=True)
            n.ins.sync_info = mybir.SyncInfo(on_wait=[w], on_update=[])
        nc.sync.drain()
        nc.all_engine_barrier()
        assert self.sems is not None
        popped = nc._tile_sem_poison_stack.pop()
        assert popped is self._sem_poison
        nc.clear_and_free_semaphores(list(self.sems.allocated().values()))
        nc.all_engine_barrier()

    tile.TileContext._drain_and_barrier = _split_drain_and_barrier
    tile.TileContext._drain_split_patch = True


def _hoist_multi_waits(nc):
    """This walrus build encodes at most ONE sync wait per instruction
    descriptor. Tile's sem assignment can put several waits on one
    instruction; hoist the extras onto same-engine no-ops inserted
    immediately before it — the engine executes them sequentially, so the
    wait semantics are unchanged."""
    import concourse.mybir as mybir

    n = 0
    for fn in nc.m.functions:
        for bb in fn.blocks:
            insts = bb.instructions
            out = []
            for ins in insts:
                si = ins.sync_info
                waits = list(si.on_wait) if si is not None else []
                if len(waits) > 1:
                    for w in waits[:-1]:
                        nop = mybir.InstNoOp(
                            name=f"I-hoistw-{nc.next_id()}",
                            engine=ins.engine,
                            ins=[],
                            outs=[],
                            sync_info=mybir.SyncInfo(on_wait=[w], on_update=[]),
                        )
                        out.append(nop)
                        n += 1
                    ins.sync_info = mybir.SyncInfo(
                        on_wait=[waits[-1]], on_update=list(si.on_update)
                    )
                out.append(ins)
            insts[:] = out
    return n


def _build_program(hoist=True):
    import concourse.bass as bass
    import concourse.mybir as mybir
    import concourse.tile as tile

    _patch_tile_drain()
    f32 = mybir.dt.float32
    f32r = mybir.dt.float32r
    Exp = mybir.ActivationFunctionType.Exp
    Copy = mybir.ActivationFunctionType.Copy

    nc = bass.Bass()
    xkvT = nc.dram_tensor("xkvT", [128, EC, T], f32r, kind="ExternalInput")
    xqT = nc.dram_tensor("xqT", [128, EC, QTILES * 128], f32r, kind="ExternalInput")
    xr = nc.dram_tensor("xr", [128, T // 128, D], f32r, kind="ExternalInput")
    wkR = nc.dram_tensor("wkR", [128, EC, D], mybir.dt.bfloat16, kind="ExternalInput")
    wqR = nc.dram_tensor("wqR", [128, EC, D], mybir.dt.bfloat16, kind="ExternalInput")
    wvT = nc.dram_tensor("wvT", [128, EC, D], f32r, kind="ExternalInput")
    ident = nc.dram_tensor("ident", [128, 128], f32, kind="ExternalInput")
    # masks are [kv, q] (transposed) here
    maska = nc.dram_tensor("maska", [128, 128], f32, kind="ExternalInput")
    maskb = nc.dram_tensor("maskb", [128, 128], f32, kind="ExternalInput")
    out_d = nc.dram_tensor("out", [QTILES * 128, D], f32, kind="ExternalOutput")

    with tile.TileContext(nc) as tc:
        with (
            tc.tile_pool(name="consts", bufs=1) as cpool,
            tc.tile_pool(name="qp", bufs=1) as qpool,
            tc.tile_pool(name="ps_st", bufs=4, space="PSUM") as ps_st,
            tc.tile_pool(name="ps_o", bufs=2, space="PSUM") as ps_o,
        ):
            # ---- W' = Wq^T.Wk on device, then Q'^T projection to SBUF ----
            # DMA order: W'-proj inputs first (first PE work), then the xq
            # stream it feeds, then constants needed later.
            qt_t = qpool.tile([128, EC, QTILES * 128], f32r, tag="qt")
            with tc.tile_pool(name="qproj", bufs=1) as qppool, \
                 tc.tile_pool(name="wpool", bufs=1) as wpool:
                bf16 = mybir.dt.bfloat16
                wk_t = wpool.tile([128, EC, D], bf16, tag="wk")
                nc.sync.dma_start(out=wk_t[:], in_=wkR[:])
                wq_t = wpool.tile([128, EC, D], bf16, tag="wq")
                nc.sync.dma_start(out=wq_t[:], in_=wqR[:])
                xq_ts = []
                for qc in range(4):
                    xq_c = qppool.tile([128, EC, 512], f32r, tag=f"xq{qc}")
                    nc.sync.dma_start(
                        out=xq_c[:], in_=xqT[:, :, qc * 512 : (qc + 1) * 512]
                    )
                    xq_ts.append(xq_c)
                id_t = cpool.tile([128, 128], f32, tag="id")
                nc.sync.dma_start(out=id_t[:], in_=ident[:])
                ma_t = cpool.tile([128, 128], f32, tag="ma")
                nc.sync.dma_start(out=ma_t[:], in_=maska[:])
                mb_t = cpool.tile([128, 128], f32, tag="mb")
                nc.sync.dma_start(out=mb_t[:], in_=maskb[:])

                # W'[d, d'] = sum_e Wq[e, d] Wk[e, d'], chunk layout
                # w2[p, m, c] = W'[m*128+p, c]
                w2_t = qppool.tile([128, EC, D], f32r, tag="w2")
                for m in range(EC):
                    pw = ps_o.tile([128, 1024], f32, tag="o", name=f"psw{m}")
                    for j in range(EC):
                        for lo, n in ((0, 512), (512, 256)):
                            nc.tensor.matmul(
                                pw[:, lo : lo + n],
                                wq_t[:, j, m * 128 : (m + 1) * 128],
                                wk_t[:, j, lo : lo + n],
                                start=(j == 0),
                                stop=(j == EC - 1),
                            )
                    nc.scalar.copy(out=w2_t[:, m, :], in_=pw[:, :D])

                # Q'^T[d', q] per chunk m: stationary w2[:, j, m-block],
                # moving xq chunks. qc-outer so S^T g0 can start early.
                for qc in range(4):
                    for m in range(EC):
                        psq = ps_st.tile(
                            [128, 512], f32, tag="st", name=f"psq{qc}_{m}"
                        )
                        for j in range(EC):
                            nc.tensor.matmul(
                                psq[:],
                                w2_t[:, j, m * 128 : (m + 1) * 128],
                                xq_ts[qc][:, j, :],
                                start=(j == 0),
                                stop=(j == EC - 1),
                            )
                        nc.scalar.copy(
                            out=qt_t[:, m, qc * 512 : (qc + 1) * 512], in_=psq[:]
                        )

            # ---- kv superblocks ----
            # oacc and Wv^T live in pools opened after the projection pools
            # close, reusing their space (pool allocation is stack-ordered).
            attn_pools = ExitStack()
            oapool = attn_pools.enter_context(tc.tile_pool(name="oacc", bufs=1))
            oacc_ts = [
                oapool.tile([128, OSTR], f32, tag=f"oacc{i}", name=f"oacc{i}")
                for i in range(QTILES)
            ]
            wv_t = oapool.tile([128, EC, D], f32r, tag="wv")
            nc.sync.dma_start(out=wv_t[:], in_=wvT[:])
            xspool = attn_pools.enter_context(tc.tile_pool(name="xs", bufs=2))
            xrpool = attn_pools.enter_context(tc.tile_pool(name="xr", bufs=2))
            ptpool = attn_pools.enter_context(tc.tile_pool(name="pt", bufs=2))
            ztpool = attn_pools.enter_context(tc.tile_pool(name="zt", bufs=2))
            spool = attn_pools.enter_context(tc.tile_pool(name="small", bufs=2))
            obpool = attn_pools.enter_context(tc.tile_pool(name="ob", bufs=2))
            def emit_retire(sb, i):
                # O = (Z.Wv^T) / l: Z^T chunks via PE transpose, then the
                # Wv^T projection, normalize on the way out.
                zt_t = ztpool.tile([128, D], f32r, tag="zt")
                for j in range(EC):
                    pz = ps_st.tile(
                        [128, 512], f32, tag="st", name=f"pz{sb}_{i}_{j}"
                    )
                    nc.tensor.transpose(
                        pz[:, 0:128],
                        oacc_ts[i][:, j * 128 : (j + 1) * 128],
                        id_t[:],
                    )
                    if j % 2 == 0:
                        nc.vector.tensor_copy(
                            out=zt_t[:, j * 128 : (j + 1) * 128],
                            in_=pz[:, 0:128],
                        )
                    else:
                        nc.scalar.copy(
                            out=zt_t[:, j * 128 : (j + 1) * 128],
                            in_=pz[:, 0:128],
                        )
                po2 = ps_o.tile([128, 1024], f32, tag="o", name=f"po2_{sb}_{i}")
                for j in range(EC):
                    for lo, n in ((0, 512), (512, 256)):
                        nc.tensor.matmul(
                            po2[:, lo : lo + n],
                            zt_t[:, j * 128 : (j + 1) * 128],
                            wv_t[:, j, lo : lo + n],
                            start=(j == 0),
                            stop=(j == EC - 1),
                        )
                recip = spool.tile([128, 1], f32, tag="recip")
                nc.vector.reciprocal(
                    out=recip[:], in_=oacc_ts[i][:, D : D + 1]
                )
                ob = obpool.tile([128, D], f32, tag="ob")
                nc.scalar.activation(ob[:], po2[:, :D], Copy, scale=recip[:, 0:1])
                nc.sync.dma_start(
                    out=out_d[i * 128 : (i + 1) * 128, :], in_=ob[:]
                )

            for sb in range(SB):
                xkv_t = xspool.tile([128, EC, SBW], f32r, tag="xs")
                nc.sync.dma_start(
                    out=xkv_t[:], in_=xkvT[:, :, sb * SBW : (sb + 1) * SBW]
                )
                # x rows for this superblock, with a ones column for l
                xr_t = xrpool.tile([128, NKT, XRW], f32r, tag="xr")
                nc.sync.dma_start(
                    out=xr_t[:, :, :D],
                    in_=xr[:, sb * NKT : (sb + 1) * NKT, :],
                )
                nc.vector.memset(xr_t[:, :, D : D + 2].bitcast(f32), 1.0)

                # ---- attention, in q-groups of up to 512 columns ----
                # active q-tiles: i in [2*sb, 16); groups start at the first
                # active tile so the terminal group is always 512 wide
                i_lo = 2 * sb
                bounds = list(range(i_lo, QTILES, 4)) + [QTILES]
                for g, ia in enumerate(bounds[:-1]):
                    ib = min(ia + 4, QTILES)   # end q-tile (exclusive)
                    qc0 = ia * 128             # first active q column
                    gw = (ib - ia) * 128       # group width (256 or 512)

                    # S^T = X.Q'^T for the group's q span, per kv-tile:
                    # stationary is the streamed x^T chunk directly.
                    stg = [
                        ps_st.tile([128, 512], f32, tag="st", name=f"st{sb}_{g}_{k}")
                        for k in range(NKT)
                    ]
                    for kt in range(NKT):
                        # q-tile 2sb never consumes kv-tiles 2,3: drop its
                        # columns there when the remainder stays >= 256 wide
                        lo = 128 if (kt >= 2 and ia == 2 * sb and gw == 512) else 0
                        for j in range(EC):
                            nc.tensor.matmul(
                                stg[kt][:, lo:gw],
                                xkv_t[:, j, kt * 128 : (kt + 1) * 128],
                                qt_t[:, j, qc0 + lo : qc0 + gw],
                                start=(j == 0),
                                stop=(j == EC - 1),
                            )
                    # causal fixups for the terminal q-tiles of this sb:
                    # q-tile 2sb terminates at kv-tiles (0,1) of this sb
                    # (tiles 2,3 are dropped from its P-matmul); q-tile
                    # 2sb+1 terminates at kv-tiles (2,3).
                    for i, kts in ((2 * sb, ((0, ma_t), (1, mb_t))),
                                   (2 * sb + 1, ((2, ma_t), (3, mb_t)))):
                        if not (ia <= i < ib):
                            continue
                        qo = i * 128 - qc0
                        for kt, m in kts:
                            nc.vector.tensor_add(
                                stg[kt][:, qo : qo + 128],
                                stg[kt][:, qo : qo + 128],
                                m[:],
                            )
                    # P^T = exp(S^T * scale) back to SBUF
                    pt_t = ptpool.tile([128, NKT, 512], f32r, tag="pt")
                    for kt in range(NKT):
                        lo = 128 if (kt >= 2 and ia == 2 * sb and gw == 512) else 0
                        nc.scalar.activation(
                            pt_t[:, kt, lo:gw], stg[kt][:, lo:gw], Exp, scale=SCALE
                        )
                    # Z += P.[X|1] per active q-tile (kv-tiles 2,3 are fully
                    # masked for q-tile 2sb on both parities: skip them)
                    for i in range(ia, ib):
                        qo = i * 128 - qc0
                        kts = (0, 1) if i == 2 * sb else (0, 1, 2, 3)
                        po = ps_o.tile([128, 1024], f32, tag="o")
                        for ki, kt in enumerate(kts):
                            lhs = pt_t[:, kt, qo : qo + 128]
                            for lo, n in ((0, 512), (512, 258)):
                                nc.tensor.matmul(
                                    po[:, lo : lo + n],
                                    lhs,
                                    xr_t[:, kt, lo : lo + n],
                                    start=(ki == 0),
                                    stop=(ki == len(kts) - 1),
                                )
                        osl = oacc_ts[i][:, : D + 1]
                        if sb == 0:
                            nc.vector.tensor_copy(out=osl, in_=po[:, : D + 1])
                        else:
                            nc.vector.tensor_add(osl, po[:, : D + 1], osl)

                    # retire the terminal q-tiles as soon as their group's
                    # merges are in; spread the two retires across groups to
                    # avoid bursting the pz ring / copy engines
                    if g == 0:
                        emit_retire(sb, 2 * sb)
                    if g == min(1, len(bounds) - 2):
                        emit_retire(sb, 2 * sb + 1)

            attn_pools.close()
    if hoist:
        _hoist_multi_waits(nc)
    return nc


def _prep_inputs(x, W_q, W_k, W_v):
    """Per-core input maps. Host-side work is layout only (transposes,
    slicing, mask construction)."""

    def chunked(a):  # [768, N] -> [128, EC, N]
        return np.ascontiguousarray(a.reshape(EC, 128, -1).transpose(1, 0, 2))

    import ml_dtypes

    wkRa = chunked(W_k).astype(ml_dtypes.bfloat16)  # Wk rows e-chunked
    wqRa = chunked(W_q).astype(ml_dtypes.bfloat16)  # Wq rows e-chunked
    wvTa = chunked(W_v.T.copy())  # Wv^T d-chunked
    ident = np.eye(128, dtype=np.float32)

    r = np.arange(128, dtype=np.float32)
    # [q, c] triangle: allowed iff c <= q; stored transposed ([kv, q])
    tri = np.where(r[None, :] <= r[:, None], 0.0, NEG).astype(np.float32)
    triT = np.ascontiguousarray(tri.T)
    zero = np.zeros((128, 128), dtype=np.float32)
    full = np.full((128, 128), NEG, dtype=np.float32)
    # per-parity (maska, maskb) for the terminal 256 kv columns
    masks_ab = [(triT, full), (zero, triT)]

    in_maps = []
    qsels = []
    for c in range(N_CORES):
        b, h = c // 2, c % 2
        xT = chunked(np.ascontiguousarray(x[b].T))  # [128, EC, T]
        xrows = np.ascontiguousarray(
            x[b].reshape(T // 128, 128, D).transpose(1, 0, 2)
        )
        qsel = np.concatenate(
            [np.arange((2 * i + h) * 128, (2 * i + h + 1) * 128) for i in range(QTILES)]
        )
        qsels.append(qsel)
        ma, mb = masks_ab[h]
        in_maps.append(
            {
                "xkvT": xT,
                "xqT": np.ascontiguousarray(xT[:, :, qsel]),
                "xr": xrows,
                "wkR": wkRa,
                "wqR": wqRa,
                "wvT": wvTa,
                "ident": ident,
                "maska": ma,
                "maskb": mb,
            }
        )
    return in_maps, qsels


def kernel(x, W_q, W_k, W_v, _trace=False):
    from concourse.bass_utils import run_bass_kernel_spmd

    if "nc" not in _CACHE:
        _CACHE["nc"] = _build_program()
    nc = _CACHE["nc"]

    in_maps, qsels = _prep_inputs(
        np.asarray(x, dtype=np.float32),
        np.asarray(W_q, dtype=np.float32),
        np.asarray(W_k, dtype=np.float32),
        np.asarray(W_v, dtype=np.float32),
    )
    res = run_bass_kernel_spmd(nc, in_maps, list(range(N_CORES)), trace=_trace)
    _CACHE["last_results"] = res

    out = np.empty((B, T, D), dtype=np.float32)
    for c in range(N_CORES):
        b = c // 2
        out[b, qsels[c]] = res.results[c]["out"]
    return out


# revision 5
# speedup vs baseline: 1.0397x; 1.0159x over previous
"""Causal self-attention (B=4, T=4096, D=768, single head, fp32) on 8 TRN2
NeuronCores — projection-free attention via reassociation.

Sharding: core <-> (batch b = core//2, parity h = core%2). Local q-tile
i = 0..15 maps to global q-tile g = 2i + h (parity interleave balances causal
work across the pair); per local q-tile i the kernel processes keys
[0, 256*(i+1)) in 8 kv superblocks of 512, masks fix up the diagonal.

Key structure (vs a direct Q/K/V-projection kernel, this removes the K and V
projections that batch-pair cores would otherwise both compute):
  - K-trick: S = Q.K^T = (X.Wq^T)(X.Wk^T)^T = (X.W').X^T, W' = Wq^T.Wk.
    W' is computed once on device from bf16 weight inputs (fp32 accumulate),
    Q' = X.W' is projected once for the local q columns, and S^T = X.Q'^T
    uses streamed x^T chunks directly as the stationary operand.
  - V-trick: O = P.V = P.(X.Wv^T) = (P.[X|1]).Wv^T. The P-matmul accumulates
    Z = P.[X|1] per q-tile across superblocks (moving operand is DMA'd x
    rows; the ones column gives the softmax denominator l in the same
    accumulation), and at retirement Z is transposed on the PE (6
    is_transpose matmuls vs identity) and projected through Wv^T once.
  - Causal skip: terminal q-tile 2sb only consumes kv-tiles {0,1} in the
    P-matmul, and its fully-masked columns are dropped from the S^T matmuls
    of kv-tiles {2,3} when the group is 512 wide.

Scheduling: q-groups start at the superblock's first active tile so the
terminal group is always 512 wide; retires are spread across the first two
groups; retire transposes alternate PSUM pools and their copies alternate
DVE/Act so the pz ring never throttles the PE; per-q-tile Z accumulators give
exact dependency tracking. All matmuls run fp32r (FP22, full PE rate,
moving >= 256); exp needs no max-subtraction (|scores| <~ 8). The Tile drain
is patched to skip the semaphore-clear epilogue (each kernel() call loads a
fresh NEFF, verified two-call safe; set KEEP_SEM_CLEAR=1 to restore).

Measured: 313.5us HW exec (was 440.3us baseline), rel err 2.1e-3 vs 2e-2
gate (bf16 weight inputs for W' account for ~2e-3; all-fp32r was 2.4e-4).
"""

import os
import sys
from contextlib import ExitStack

import numpy as np

if "/opt/trn_rl_repo" not in sys.path:
    sys.path.insert(0, "/opt/trn_rl_repo")

B, T, D = 4, 4096, 768
N_CORES = 8
QTILES = 16          # local q-tiles per core, 128 rows each
EC = D // 128        # 6 d/e chunks of 128
SB = 8               # kv superblocks
SBW = 512            # superblock width (keys)
NKT = SBW // 128     # kv 128-tiles per superblock
OSTR = D + 4         # oacc slot stride: [Z 768 | l 1 | pad 3]
XRW = D + 4          # xr row-tile width: [X 768 | ones 2 | pad 2]
NEG = -1.0e9
SCALE = 1.0 / float(np.sqrt(D))

_CACHE = {}


def _patch_tile_drain():
    """This walrus build accepts only one sync wait per instruction;
    TileContext's tail drain carries one wait per outstanding proc. Split
    them onto individual SP no-ops (SP executes sequentially, so semantics
    are unchanged)."""
    import concourse.mybir as mybir
    import concourse.tile as tile
    from concourse.vector_clock import ScopedClock

    if getattr(tile.TileContext, "_drain_split_patch", False):
        return

    def _split_drain_and_barrier(self, tick_clock, wait_clock):
        nc = self.nc
        carrier = nc.sync.nop(nofuse=True)
        wait_clock.add_sem_waits(
            carrier.ins, ScopedClock({None: tick_clock.global_clock})
        )
        si = carrier.ins.sync_info
        waits = list(si.on_wait) if si is not None else []
        carrier.ins.sync_info = mybir.SyncInfo(on_wait=waits[:1], on_update=[])
        for w in waits[1:]:
            n = nc.sync.nop(nofuse=True)
            n.ins.sync_info = mybir.SyncInfo(on_wait=[w], on_update=[])
        nc.sync.drain()
        nc.all_engine_barrier()
        assert self.sems is not None
        popped = nc._tile_sem_poison_stack.pop()
        assert popped is self._sem_poison
        if os.environ.get("KEEP_SEM_CLEAR"):
            nc.clear_and_free_semaphores(list(self.sems.allocated().values()))
            nc.all_engine_barrier()

    tile.TileContext._drain_and_barrier = _split_drain_and_barrier
    tile.TileContext._drain_split_patch = True


def _hoist_multi_waits(nc):
    """This walrus build encodes at most ONE sync wait per instruction
    descriptor. Tile's sem assignment can put several waits on one
    instruction; hoist the extras onto same-engine no-ops inserted
    immediately before it — the engine executes them sequentially, so the
    wait semantics are unchanged."""
    import concourse.mybir as mybir

    n = 0
    for fn in nc.m.functions:
        for bb in fn.blocks:
            insts = bb.instructions
            out = []
            for ins in insts:
                si = ins.sync_info
                waits = list(si.on_wait) if si is not None else []
                if len(waits) > 1:
                    for w in waits[:-1]:
                        nop = mybir.InstNoOp(
                            name=f"I-hoistw-{nc.next_id()}",
                            engine=ins.engine,
                            ins=[],
                            outs=[],
                            sync_info=mybir.SyncInfo(on_wait=[w], on_update=[]),
                        )
                        out.append(nop)
                        n += 1
                    ins.sync_info = mybir.SyncInfo(
                        on_wait=[waits[-1]], on_update=list(si.on_update)
                    )
                out.append(ins)
            insts[:] = out
    return n


def _build_program(hoist=True):
    import concourse.bass as bass
    import concourse.mybir as mybir
    import concourse.tile as tile

    _patch_tile_drain()
    f32 = mybir.dt.float32
    f32r = mybir.dt.float32r
    Exp = mybir.ActivationFunctionType.Exp
    Copy = mybir.ActivationFunctionType.Copy

    nc = bass.Bass()
    xkvT = nc.dram_tensor("xkvT", [128, EC, T], f32r, kind="ExternalInput")
    xqT = nc.dram_tensor("xqT", [128, EC, QTILES * 128], f32r, kind="ExternalInput")
    xr = nc.dram_tensor("xr", [128, T // 128, D], f32r, kind="ExternalInput")
    wkR = nc.dram_tensor("wkR", [128, EC, D], mybir.dt.bfloat16, kind="ExternalInput")
    wqR = nc.dram_tensor("wqR", [128, EC, D], mybir.dt.bfloat16, kind="ExternalInput")
    wvT = nc.dram_tensor("wvT", [128, EC, D], f32r, kind="ExternalInput")
    ident = nc.dram_tensor("ident", [128, 128], f32, kind="ExternalInput")
    # masks are [kv, q] (transposed) here
    maska = nc.dram_tensor("maska", [128, 128], f32, kind="ExternalInput")
    maskb = nc.dram_tensor("maskb", [128, 128], f32, kind="ExternalInput")
    out_d = nc.dram_tensor("out", [QTILES * 128, D], f32, kind="ExternalOutput")

    with tile.TileContext(nc) as tc:
        with (
            tc.tile_pool(name="consts", bufs=1) as cpool,
            tc.tile_pool(name="qp", bufs=1) as qpool,
            tc.tile_pool(name="ps_st", bufs=4, space="PSUM") as ps_st,
            tc.tile_pool(name="ps_o", bufs=2, space="PSUM") as ps_o,
        ):
            # ---- W' = Wq^T.Wk on device, then Q'^T projection to SBUF ----
            # DMA order: W'-proj inputs first (first PE work), then the xq
            # stream it feeds, then constants needed later.
            qt_t = qpool.tile([128, EC, QTILES * 128], f32r, tag="qt")
            with tc.tile_pool(name="qproj", bufs=1) as qppool, \
                 tc.tile_pool(name="wpool", bufs=1) as wpool:
                bf16 = mybir.dt.bfloat16
                wk_t = wpool.tile([128, EC, D], bf16, tag="wk")
                nc.sync.dma_start(out=wk_t[:], in_=wkR[:])
                wq_t = wpool.tile([128, EC, D], bf16, tag="wq")
                nc.sync.dma_start(out=wq_t[:], in_=wqR[:])
                xq_ts = []
                for qc in range(4):
                    xq_c = qppool.tile([128, EC, 512], f32r, tag=f"xq{qc}")
                    nc.sync.dma_start(
                        out=xq_c[:], in_=xqT[:, :, qc * 512 : (qc + 1) * 512]
                    )
                    xq_ts.append(xq_c)
                id_t = cpool.tile([128, 128], f32, tag="id")
                nc.sync.dma_start(out=id_t[:], in_=ident[:])
                ma_t = cpool.tile([128, 128], f32, tag="ma")
                nc.sync.dma_start(out=ma_t[:], in_=maska[:])
                mb_t = cpool.tile([128, 128], f32, tag="mb")
                nc.sync.dma_start(out=mb_t[:], in_=maskb[:])

                # W'[d, d'] = sum_e Wq[e, d] Wk[e, d'], chunk layout
                # w2[p, m, c] = W'[m*128+p, c]
                w2_t = qppool.tile([128, EC, D], f32r, tag="w2")
                for m in range(EC):
                    pw = ps_o.tile([128, 1024], f32, tag="o", name=f"psw{m}")
                    for j in range(EC):
                        for lo, n in ((0, 512), (512, 256)):
                            nc.tensor.matmul(
                                pw[:, lo : lo + n],
                                wq_t[:, j, m * 128 : (m + 1) * 128],
                                wk_t[:, j, lo : lo + n],
                                start=(j == 0),
                                stop=(j == EC - 1),
                            )
                    nc.scalar.copy(out=w2_t[:, m, :], in_=pw[:, :D])

                # Q'^T[d', q] per chunk m: stationary w2[:, j, m-block],
                # moving xq chunks. qc-outer so S^T g0 can start early.
                for qc in range(4):
                    for m in range(EC):
                        psq = ps_st.tile(
                            [128, 512], f32, tag="st", name=f"psq{qc}_{m}"
                        )
                        for j in range(EC):
                            nc.tensor.matmul(
                                psq[:],
                                w2_t[:, j, m * 128 : (m + 1) * 128],
                                xq_ts[qc][:, j, :],
                                start=(j == 0),
                                stop=(j == EC - 1),
                            )
                        nc.scalar.copy(
                            out=qt_t[:, m, qc * 512 : (qc + 1) * 512], in_=psq[:]
                        )

            # ---- kv superblocks ----
            # oacc and Wv^T live in pools opened after the projection pools
            # close, reusing their space (pool allocation is stack-ordered).
            attn_pools = ExitStack()
            oapool = attn_pools.enter_context(tc.tile_pool(name="oacc", bufs=1))
            oacc_ts = [
                oapool.tile([128, OSTR], f32, tag=f"oacc{i}", name=f"oacc{i}")
                for i in range(QTILES)
            ]
            wv_t = oapool.tile([128, EC, D], f32r, tag="wv")
            nc.sync.dma_start(out=wv_t[:], in_=wvT[:])
            xspool = attn_pools.enter_context(tc.tile_pool(name="xs", bufs=2))
            xrpool = attn_pools.enter_context(tc.tile_pool(name="xr", bufs=2))
            ptpool = attn_pools.enter_context(tc.tile_pool(name="pt", bufs=2))
            ztpool = attn_pools.enter_context(tc.tile_pool(name="zt", bufs=2))
            spool = attn_pools.enter_context(tc.tile_pool(name="small", bufs=2))
            obpool = attn_pools.enter_context(tc.tile_pool(name="ob", bufs=2))
            def emit_retire(sb, i):
                # O = (Z.Wv^T) / l: Z^T chunks via PE transpose, then the
                # Wv^T projection, normalize on the way out.
                zt_t = ztpool.tile([128, D], f32r, tag="zt")
                for j in range(EC):
                    if j % 2 == 0:
                        pz = ps_st.tile(
                            [128, 512], f32, tag="st", name=f"pz{sb}_{i}_{j}"
                        )
                    else:
                        pz = ps_o.tile(
                            [128, 1024], f32, tag="o", name=f"pzo{sb}_{i}_{j}"
                        )
                    nc.tensor.transpose(
                        pz[:, 0:128],
                        oacc_ts[i][:, j * 128 : (j + 1) * 128],
                        id_t[:],
                    )
                    if j % 2 == 0:
                        nc.vector.tensor_copy(
                            out=zt_t[:, j * 128 : (j + 1) * 128],
                            in_=pz[:, 0:128],
                        )
                    else:
                        nc.scalar.copy(
                            out=zt_t[:, j * 128 : (j + 1) * 128],
                            in_=pz[:, 0:128],
                        )
                po2 = ps_o.tile([128, 1024], f32, tag="o", name=f"po2_{sb}_{i}")
                for j in range(EC):
                    for lo, n in ((0, 512), (512, 256)):
                        nc.tensor.matmul(
                            po2[:, lo : lo + n],
                            zt_t[:, j * 128 : (j + 1) * 128],
                            wv_t[:, j, lo : lo + n],
                            start=(j == 0),
                            stop=(j == EC - 1),
                        )
                recip = spool.tile([128, 1], f32, tag="recip")
                nc.vector.reciprocal(
                    out=recip[:], in_=oacc_ts[i][:, D : D + 1]
                )
                ob = obpool.tile([128, D], f32, tag="ob")
                nc.scalar.activation(ob[:], po2[:, :D], Copy, scale=recip[:, 0:1])
                nc.sync.dma_start(
                    out=out_d[i * 128 : (i + 1) * 128, :], in_=ob[:]
                )

            for sb in range(SB):
                xkv_t = xspool.tile([128, EC, SBW], f32r, tag="xs")
                nc.sync.dma_start(
                    out=xkv_t[:], in_=xkvT[:, :, sb * SBW : (sb + 1) * SBW]
                )
                # x rows for this superblock, with a ones column for l
                xr_t = xrpool.tile([128, NKT, XRW], f32r, tag="xr")
                nc.sync.dma_start(
                    out=xr_t[:, :, :D],
                    in_=xr[:, sb * NKT : (sb + 1) * NKT, :],
                )
                nc.vector.memset(xr_t[:, :, D : D + 2].bitcast(f32), 1.0)

                # ---- attention, in q-groups of up to 512 columns ----
                # active q-tiles: i in [2*sb, 16); groups start at the first
                # active tile so the terminal group is always 512 wide
                i_lo = 2 * sb
                bounds = list(range(i_lo, QTILES, 4)) + [QTILES]
                for g, ia in enumerate(bounds[:-1]):
                    ib = min(ia + 4, QTILES)   # end q-tile (exclusive)
                    qc0 = ia * 128             # first active q column
                    gw = (ib - ia) * 128       # group width (256 or 512)

                    # S^T = X.Q'^T for the group's q span, per kv-tile:
                    # stationary is the streamed x^T chunk directly.
                    stg = [
                        ps_st.tile([128, 512], f32, tag="st", name=f"st{sb}_{g}_{k}")
                        for k in range(NKT)
                    ]
                    for kt in range(NKT):
                        # q-tile 2sb never consumes kv-tiles 2,3: drop its
                        # columns there when the remainder stays >= 256 wide
                        lo = 128 if (kt >= 2 and ia == 2 * sb and gw == 512) else 0
                        for j in range(EC):
                            nc.tensor.matmul(
                                stg[kt][:, lo:gw],
                                xkv_t[:, j, kt * 128 : (kt + 1) * 128],
                                qt_t[:, j, qc0 + lo : qc0 + gw],
                                start=(j == 0),
                                stop=(j == EC - 1),
                            )
                    # causal fixups for the terminal q-tiles of this sb:
                    # q-tile 2sb terminates at kv-tiles (0,1) of this sb
                    # (tiles 2,3 are dropped from its P-matmul); q-tile
                    # 2sb+1 terminates at kv-tiles (2,3).
                    for i, kts in ((2 * sb, ((0, ma_t), (1, mb_t))),
                                   (2 * sb + 1, ((2, ma_t), (3, mb_t)))):
                        if not (ia <= i < ib):
                            continue
                        qo = i * 128 - qc0
                        for kt, m in kts:
                            nc.vector.tensor_add(
                                stg[kt][:, qo : qo + 128],
                                stg[kt][:, qo : qo + 128],
                                m[:],
                            )
                    # P^T = exp(S^T * scale) back to SBUF
                    pt_t = ptpool.tile([128, NKT, 512], f32r, tag="pt")
                    for kt in range(NKT):
                        lo = 128 if (kt >= 2 and ia == 2 * sb and gw == 512) else 0
                        nc.scalar.activation(
                            pt_t[:, kt, lo:gw], stg[kt][:, lo:gw], Exp, scale=SCALE
                        )
                    # Z += P.[X|1] per active q-tile (kv-tiles 2,3 are fully
                    # masked for q-tile 2sb on both parities: skip them)
                    for i in range(ia, ib):
                        qo = i * 128 - qc0
                        kts = (0, 1) if i == 2 * sb else (0, 1, 2, 3)
                        po = ps_o.tile([128, 1024], f32, tag="o")
                        for ki, kt in enumerate(kts):
                            lhs = pt_t[:, kt, qo : qo + 128]
                            for lo, n in ((0, 512), (512, 258)):
                                nc.tensor.matmul(
                                    po[:, lo : lo + n],
                                    lhs,
                                    xr_t[:, kt, lo : lo + n],
                                    start=(ki == 0),
                                    stop=(ki == len(kts) - 1),
                                )
                        osl = oacc_ts[i][:, : D + 1]
                        if sb == 0:
                            nc.vector.tensor_copy(out=osl, in_=po[:, : D + 1])
                        else:
                            nc.vector.tensor_add(osl, po[:, : D + 1], osl)

                    # retire the terminal q-tiles as soon as their group's
                    # merges are in; spread the two retires across groups to
                    # avoid bursting the pz ring / copy engines
                    if g == 0:
                        emit_retire(sb, 2 * sb)
                    if g == min(1, len(bounds) - 2):
                        emit_retire(sb, 2 * sb + 1)

            attn_pools.close()
    if hoist:
        _hoist_multi_waits(nc)
    return nc


def _prep_inputs(x, W_q, W_k, W_v):
    """Per-core input maps. Host-side work is layout only (transposes,
    slicing, mask construction)."""

    def chunked(a):  # [768, N] -> [128, EC, N]
        return np.ascontiguousarray(a.reshape(EC, 128, -1).transpose(1, 0, 2))

    import ml_dtypes

    wkRa = chunked(W_k).astype(ml_dtypes.bfloat16)  # Wk rows e-chunked
    wqRa = chunked(W_q).astype(ml_dtypes.bfloat16)  # Wq rows e-chunked
    wvTa = chunked(W_v.T.copy())  # Wv^T d-chunked
    ident = np.eye(128, dtype=np.float32)

    r = np.arange(128, dtype=np.float32)
    # [q, c] triangle: allowed iff c <= q; stored transposed ([kv, q])
    tri = np.where(r[None, :] <= r[:, None], 0.0, NEG).astype(np.float32)
    triT = np.ascontiguousarray(tri.T)
    zero = np.zeros((128, 128), dtype=np.float32)
    full = np.full((128, 128), NEG, dtype=np.float32)
    # per-parity (maska, maskb) for the terminal 256 kv columns
    masks_ab = [(triT, full), (zero, triT)]

    in_maps = []
    qsels = []
    for c in range(N_CORES):
        b, h = c // 2, c % 2
        xT = chunked(np.ascontiguousarray(x[b].T))  # [128, EC, T]
        xrows = np.ascontiguousarray(
            x[b].reshape(T // 128, 128, D).transpose(1, 0, 2)
        )
        qsel = np.concatenate(
            [np.arange((2 * i + h) * 128, (2 * i + h + 1) * 128) for i in range(QTILES)]
        )
        qsels.append(qsel)
        ma, mb = masks_ab[h]
        in_maps.append(
            {
                "xkvT": xT,
                "xqT": np.ascontiguousarray(xT[:, :, qsel]),
                "xr": xrows,
                "wkR": wkRa,
                "wqR": wqRa,
                "wvT": wvTa,
                "ident": ident,
                "maska": ma,
                "maskb": mb,
            }
        )
    return in_maps, qsels


def kernel(x, W_q, W_k, W_v, _trace=False):
    from concourse.bass_utils import run_bass_kernel_spmd

    if "nc" not in _CACHE:
        _CACHE["nc"] = _build_program()
    nc = _CACHE["nc"]

    in_maps, qsels = _prep_inputs(
        np.asarray(x, dtype=np.float32),
        np.asarray(W_q, dtype=np.float32),
        np.asarray(W_k, dtype=np.float32),
        np.asarray(W_v, dtype=np.float32),
    )
    res = run_bass_kernel_spmd(nc, in_maps, list(range(N_CORES)), trace=_trace)
    _CACHE["last_results"] = res

    out = np.empty((B, T, D), dtype=np.float32)
    for c in range(N_CORES):
        b = c // 2
        out[b, qsels[c]] = res.results[c]["out"]
    return out
